# revision 24
# baseline (speedup 1.0000x reference)
"""YOLOv3-style detection decode kernel for Trainium2 (8 NeuronCores).

Data-parallel over batch (32 -> 4 per core). Host marshals each core's head
tensors into a cells-on-partitions layout x[p, (b k a c)] (cell = k*128+p,
c = 85 attrs per anchor); since 3 anchors * 85 = 255 = the channel count,
(b, k, a) collapse into one free dim Z and the device needs no transposes:

  - argmax over the 80 classes per (cell, anchor) via two segmented DVE
    reductions: phase-maxes p8[j] = max_g x[8g+j] and group-maxes
    q10[g] = max_j x[8g+j] (one tensor_reduce each per scale). The class
    index is 8*g* + j*, with g*/j* recovered by an is_ge-against-max
    compare and a descending-weight max (ties break toward the FIRST
    index, matching jnp.argmax).
  - box decode reads strided views of the same tiles (exp/scale on ACT).
  - outputs are packed [p, b, k, a, 6] per scale; the host re-interleaves.
"""

import sys

import numpy as np

if "/opt/trn_rl_repo" not in sys.path:
    sys.path.insert(0, "/opt/trn_rl_repo")

NUM_ATTRS = 85
B_LOC = 4  # batches per core (32 / 8)
N_CORES = 8

# (name, H, stride)
_SCALES = (
    ("13", 13, 32.0),
    ("26", 26, 16.0),
    ("52", 52, 8.0),
)


def _scale_cfg():
    cfgs = []
    for name, H, stride in _SCALES:
        HW = H * H
        nblk = -(-HW // 128)
        cfgs.append(dict(name=name, H=H, W=H, HW=HW, stride=stride,
                         nblk=nblk, HWp=nblk * 128))
    return cfgs


SCFG = _scale_cfg()

# consts layout: w8(8) | w10(10) | thr(1) | per scale: gx4(4nb) gy4(4nb) anch(6)
_CST_W8 = 0
_CST_W10 = 8
_CST_THR = 18
_CST_SC = {}
_off = 19
for _s in SCFG:
    _CST_SC[_s["name"]] = _off
    _off += 8 * _s["nblk"] + 6
CST_COLS = _off


def _build_program():
    import concourse.bass as bass
    import concourse.mybir as mybir
    from concourse.tile import TileContext

    f32 = mybir.dt.float32
    bf16 = mybir.dt.bfloat16
    Alu = mybir.AluOpType
    Act = mybir.ActivationFunctionType
    X = mybir.AxisListType.X

    nc = bass.Bass(trn_type="TRN2")

    xin = {}
    opk = {}
    for s in SCFG:
        n = s["name"]
        xin[n] = nc.declare_dram_parameter(
            f"x{n}", [128, B_LOC * s["nblk"] * 255], f32, False)
        opk[n] = nc.declare_dram_parameter(
            f"opack{n}", [128, B_LOC * s["nblk"] * 18], f32, True)
    cst_p = nc.declare_dram_parameter("cst", [128, CST_COLS], f32, False)

    with TileContext(nc) as tc:
        from contextlib import ExitStack
        with ExitStack() as ctx:
            cpool = ctx.enter_context(tc.tile_pool(name="consts", bufs=1))
            x52pool = ctx.enter_context(tc.tile_pool(name="x52", bufs=2))
            # fold scratch: consumed only by the (serial) DVE queue, so a
            # single buffer per tag costs no parallelism
            fpool = ctx.enter_context(tc.tile_pool(name="folds", bufs=1))

            # x13 first: its consumers idle at kernel start; consts are only
            # needed later (mask/cx), so they stream second.
            xt13 = cpool.tile([128, B_LOC * SCFG[0]["nblk"] * 255], f32,
                              tag="xt13", name="xt13")
            nc.sync.dma_start(out=xt13[:, :], in_=xin["13"][:, :])
            cstt = cpool.tile([128, CST_COLS], f32, tag="cst", name="cstt")
            nc.sync.dma_start(out=cstt[:, :], in_=cst_p[:, :])
            thr = cstt[:, _CST_THR:_CST_THR + 1]

            # bf16 copies of the index weights (2-byte dtype enables the
            # DVE 2x mode for the small extraction ops)
            w8c = cpool.tile([128, 8], bf16, tag="w8c", name="w8c")
            nc.vector.tensor_copy(out=w8c[:, :],
                                  in_=cstt[:, _CST_W8:_CST_W8 + 8])
            w10c = cpool.tile([128, 10], bf16, tag="w10c", name="w10c")
            nc.vector.tensor_copy(out=w10c[:, :],
                                  in_=cstt[:, _CST_W10:_CST_W10 + 10])

            def scale_tiles(s):
                n, nb = s["name"], s["nblk"]
                Z = B_LOC * nb * 3
                t = {}
                for key, w in (("p8", 8), ("q10", 10), ("m", 1),
                               ("ts", 1), ("mask", 1), ("ex", 2), ("wh", 2),
                               ("cx", 1), ("cy", 1), ("opk", 6)):
                    t[key] = cpool.tile([128, Z * w], f32, tag=f"{key}{n}",
                                        name=f"{key}{n}")
                for key, w in (("eq8", 8), ("eq10", 10), ("ew8", 8),
                               ("ew10", 10), ("r8", 1), ("r10", 1),
                               ("rs8", 4), ("rs10", 5)):
                    t[key] = cpool.tile([128, Z * w], bf16, tag=f"{key}{n}",
                                        name=f"{key}{n}")
                t["m2"] = cpool.tile([128, Z * 4], f32, tag=f"m2{n}",
                                     name=f"m2{n}")
                t["bx"] = cpool.tile([128, Z * 4], f32, tag=f"bx{n}",
                                     name=f"bx{n}")
                return t

            def class_reduces(s, t, xtv, b):
                """Phase/group reduces + conf + cx/cy for one input view.

                xtv: [p, z, c=85] view (z spans (k a) or (b k a)); b is None
                for the all-batch case, else the batch index of a transient
                tile.
                """
                n, nb, stride = s["name"], s["nblk"], s["stride"]
                off = _CST_SC[n]
                zc = xtv.shape[1]  # cells*anchors covered by this view
                lo = 0 if b is None else b * nb * 3
                hi = lo + zc

                cls = xtv[:, :, 5:85]
                p8v = t["p8"][:, lo * 8:hi * 8].rearrange(
                    "p (z j) -> p z j", j=8)
                q10v = t["q10"][:, lo * 10:hi * 10].rearrange(
                    "p (z g) -> p z g", g=10)

                # Tournament folds (tensor_tensor reads two streams/cycle,
                # so folds cost half a single-stream tensor_reduce; all
                # slices are stride-1 in the inner dim).
                # phase-max p8[j] = max_g cls[8g+j]: fold the group axis.
                f1 = fpool.tile([128, zc * 40], f32, tag="f1", name="f1")
                f1v = f1[:, :].rearrange("p (z c) -> p z c", c=40)
                nc.vector.tensor_tensor(out=f1v, in0=cls[:, :, 0:40],
                                        in1=cls[:, :, 40:80], op=Alu.max)
                f2 = fpool.tile([128, zc * 16], f32, tag="f2", name="f2")
                f2v = f2[:, :].rearrange("p (z c) -> p z c", c=16)
                nc.vector.tensor_tensor(out=f2v, in0=f1v[:, :, 0:16],
                                        in1=f1v[:, :, 16:32], op=Alu.max)
                f3 = fpool.tile([128, zc * 8], f32, tag="f3", name="f3")
                f3v = f3[:, :].rearrange("p (z c) -> p z c", c=8)
                nc.vector.tensor_tensor(out=f3v, in0=f2v[:, :, 0:8],
                                        in1=f2v[:, :, 8:16], op=Alu.max)
                nc.vector.tensor_tensor(out=p8v, in0=f3v,
                                        in1=f1v[:, :, 32:40], op=Alu.max)
                # group-max q10[g] = max_j cls[8g+j]: fold within groups.
                cg = cls.rearrange("p z (g j) -> p z g j", g=10, j=8)
                y1 = fpool.tile([128, zc * 40], f32, tag="y1", name="y1")
                y1v = y1[:, :].rearrange("p (z g j) -> p z g j", g=10, j=4)
                nc.vector.tensor_tensor(out=y1v, in0=cg[:, :, :, 0:4],
                                        in1=cg[:, :, :, 4:8], op=Alu.max)
                y1z = y1[:, :].rearrange("p (w j) -> p w j", j=4)
                y2 = fpool.tile([128, zc * 20], f32, tag="y2", name="y2")
                y2z = y2[:, :].rearrange("p (w j) -> p w j", j=2)
                nc.vector.tensor_tensor(out=y2z, in0=y1z[:, :, 0:2],
                                        in1=y1z[:, :, 2:4], op=Alu.max)
                nc.vector.tensor_tensor(
                    out=t["q10"][:, lo * 10:hi * 10],
                    in0=y2z[:, :, 0:1].squeeze(2),
                    in1=y2z[:, :, 1:2].squeeze(2), op=Alu.max)

                conf = xtv[:, :, 0:1].squeeze(2)
                maskv = t["mask"][:, lo:hi]
                nc.vector.tensor_single_scalar(
                    out=maskv, in_=conf, scalar=thr, op=Alu.is_gt)
                opkz = t["opk"][:, lo * 6:hi * 6].rearrange(
                    "p (z q) -> p z q", q=6)
                nc.vector.tensor_tensor(
                    out=opkz[:, :, 0:1].squeeze(2), in0=conf, in1=maskv,
                    op=Alu.mult)

                # exp(tw,th) on ACT; cx/cy on DVE
                exv = t["ex"][:, lo * 2:hi * 2].rearrange(
                    "p (z e) -> p z e", e=2)
                nc.scalar.activation(out=exv, in_=xtv[:, :, 3:5],
                                     func=Act.Exp)
                # gx4/gy4 are host-replicated over batches, so a [p, zc/3, 3]
                # broadcast view always lines up with this z range.
                gx = cstt[:, off + lo // 3:off + hi // 3]
                gy = cstt[:, off + 4 * nb + lo // 3:off + 4 * nb + hi // 3]
                gxb = gx.unsqueeze(2).broadcast_to([128, zc // 3, 3])
                gyb = gy.unsqueeze(2).broadcast_to([128, zc // 3, 3])
                tx = xtv[:, :, 1:2].squeeze(2).rearrange(
                    "p (w a) -> p w a", a=3)
                ty = xtv[:, :, 2:3].squeeze(2).rearrange(
                    "p (w a) -> p w a", a=3)
                cxv = t["cx"][:, lo:hi].rearrange("p (w a) -> p w a", a=3)
                cyv = t["cy"][:, lo:hi].rearrange("p (w a) -> p w a", a=3)
                nc.vector.scalar_tensor_tensor(
                    out=cxv, in0=tx, scalar=stride, in1=gxb,
                    op0=Alu.mult, op1=Alu.add)
                nc.vector.scalar_tensor_tensor(
                    out=cyv, in0=ty, scalar=stride, in1=gyb,
                    op0=Alu.mult, op1=Alu.add)

            def per_scale(s, t):
                """All-batch epilogue on compact scratch tiles."""
                n, nb = s["name"], s["nblk"]
                off = _CST_SC[n]
                anch = cstt[:, off + 8 * nb:off + 8 * nb + 6]
                Z = B_LOC * nb * 3

                def zv(tile, w):
                    return tile[:, :].rearrange("p (z q) -> p z q", q=w)

                p8 = zv(t["p8"], 8)
                q10 = zv(t["q10"], 10)
                eq8 = zv(t["eq8"], 8)
                eq10 = zv(t["eq10"], 10)
                ew8 = zv(t["ew8"], 8)
                ew10 = zv(t["ew10"], 10)
                m = t["m"][:, :]
                r8 = t["r8"][:, :]
                r10 = t["r10"][:, :]
                tsv = t["ts"][:, :]
                mask = t["mask"][:, :]
                opkq = zv(t["opk"], 6)

                # m = max over phases, via TT folds (8 -> 4 -> 2 -> 1)
                m2 = zv(t["m2"], 4)
                nc.vector.tensor_tensor(out=m2, in0=p8[:, :, 0:4],
                                        in1=p8[:, :, 4:8], op=Alu.max)
                m2z = t["m2"][:, :].rearrange("p (w c) -> p w c", c=2)
                nc.vector.tensor_tensor(out=m2z[:, :, 0:1].squeeze(2),
                                        in0=m2z[:, :, 0:1].squeeze(2),
                                        in1=m2z[:, :, 1:2].squeeze(2),
                                        op=Alu.max)
                m4 = zv(t["m2"], 4)
                nc.vector.tensor_tensor(out=m, in0=m4[:, :, 0:1].squeeze(2),
                                        in1=m4[:, :, 2:3].squeeze(2),
                                        op=Alu.max)
                mb8 = m.unsqueeze(2).broadcast_to([128, Z, 8])
                mb10 = m.unsqueeze(2).broadcast_to([128, Z, 10])
                w8b = w8c[:, :].unsqueeze(1).broadcast_to([128, Z, 8])
                w10b = w10c[:, :].unsqueeze(1).broadcast_to([128, Z, 10])

                # j* / g* via descending-weight max (first-index tie-break);
                # eq/weights/r are exact small values -> bf16 (2x DVE mode)
                nc.vector.tensor_tensor(out=eq8, in0=p8, in1=mb8,
                                        op=Alu.is_ge)
                nc.vector.tensor_tensor(out=ew8, in0=eq8, in1=w8b,
                                        op=Alu.mult)
                rs8 = zv(t["rs8"], 4)
                nc.vector.tensor_tensor(out=rs8, in0=ew8[:, :, 0:4],
                                        in1=ew8[:, :, 4:8], op=Alu.max)
                rs8z = t["rs8"][:, :].rearrange("p (w c) -> p w c", c=2)
                nc.vector.tensor_tensor(out=rs8z[:, :, 0:1].squeeze(2),
                                        in0=rs8z[:, :, 0:1].squeeze(2),
                                        in1=rs8z[:, :, 1:2].squeeze(2),
                                        op=Alu.max)
                nc.vector.tensor_tensor(out=r8, in0=rs8[:, :, 0:1].squeeze(2),
                                        in1=rs8[:, :, 2:3].squeeze(2),
                                        op=Alu.max)
                nc.vector.tensor_tensor(out=eq10, in0=q10, in1=mb10,
                                        op=Alu.is_ge)
                nc.vector.tensor_tensor(out=ew10, in0=eq10, in1=w10b,
                                        op=Alu.mult)
                rs10 = zv(t["rs10"], 5)
                nc.vector.tensor_tensor(out=rs10, in0=ew10[:, :, 0:5],
                                        in1=ew10[:, :, 5:10], op=Alu.max)
                nc.vector.tensor_tensor(out=rs10[:, :, 0:2],
                                        in0=rs10[:, :, 0:2],
                                        in1=rs10[:, :, 2:4], op=Alu.max)
                nc.vector.tensor_tensor(out=rs10[:, :, 0:1].squeeze(2),
                                        in0=rs10[:, :, 0:1].squeeze(2),
                                        in1=rs10[:, :, 1:2].squeeze(2),
                                        op=Alu.max)
                nc.vector.tensor_tensor(out=r10,
                                        in0=rs10[:, :, 0:1].squeeze(2),
                                        in1=rs10[:, :, 4:5].squeeze(2),
                                        op=Alu.max)
                # idx = 88 - 8*r10 - r8 ; cls_m = (ts + 88) * mask
                nc.vector.scalar_tensor_tensor(
                    out=tsv, in0=r10, scalar=-8.0, in1=r8,
                    op0=Alu.mult, op1=Alu.subtract)
                nc.vector.scalar_tensor_tensor(
                    out=opkq[:, :, 5:6].squeeze(2), in0=tsv, scalar=88.0,
                    in1=mask, op0=Alu.add, op1=Alu.mult)

                # boxes: wh = anch * exp (ACT) -> x1/y1/x2/y2 -> masked
                exw = t["ex"][:, :].rearrange("p (w a e) -> p w a e",
                                              a=3, e=2)
                whw = t["wh"][:, :].rearrange("p (w a e) -> p w a e",
                                              a=3, e=2)
                for a in range(3):
                    for q in range(2):
                        nc.scalar.activation(
                            out=whw[:, :, a, q:q + 1].squeeze(2),
                            in_=exw[:, :, a, q:q + 1].squeeze(2),
                            func=Act.Copy,
                            scale=anch[:, 2 * a + q:2 * a + q + 1])
                whz = zv(t["wh"], 2)
                wx = whz[:, :, 0:1].squeeze(2)
                wy = whz[:, :, 1:2].squeeze(2)
                cx = t["cx"][:, :]
                cy = t["cy"][:, :]
                bx = zv(t["bx"], 4)
                for q, (wv, cv, sc) in enumerate(
                        ((wx, cx, -0.5), (wy, cy, -0.5),
                         (wx, cx, 0.5), (wy, cy, 0.5))):
                    nc.vector.scalar_tensor_tensor(
                        out=bx[:, :, q:q + 1].squeeze(2), in0=wv, scalar=sc,
                        in1=cv, op0=Alu.mult, op1=Alu.add)
                mb4 = mask.unsqueeze(2).broadcast_to([128, Z, 4])
                nc.vector.tensor_tensor(out=opkq[:, :, 1:5], in0=bx,
                                        in1=mb4, op=Alu.mult)
                nc.sync.dma_start(out=opk[n][:, :], in_=t["opk"][:, :])

            # ---- scales 13 / 26: one static tile, all batches at once ----
            for s in SCFG[:2]:
                n = s["name"]
                t = scale_tiles(s)
                if n == "13":
                    xt = xt13
                else:
                    xt = cpool.tile([128, B_LOC * s["nblk"] * 255], f32,
                                    tag=f"xt{n}", name=f"xt{n}")
                    nc.sync.dma_start(out=xt[:, :], in_=xin[n][:, :])
                vz = xt[:, :].rearrange("p (z c) -> p z c", c=85)
                class_reduces(s, t, vz, None)
                per_scale(s, t)

            # ---- scale 52: per-batch pipelined input tiles ----
            s = SCFG[2]
            t = scale_tiles(s)
            nb = s["nblk"]
            for b in range(B_LOC):
                xtb = x52pool.tile([128, nb * 255], f32, tag="x52",
                                   name="x52b")
                nc.sync.dma_start(
                    out=xtb[:, :],
                    in_=xin[s["name"]][:, b * nb * 255:(b + 1) * nb * 255])
                vz = xtb[:, :].rearrange("p (z c) -> p z c", c=85)
                class_reduces(s, t, vz, b)
            per_scale(s, t)

    return nc


def _split_sync_waits(nc, limit=1):
    """Move overflow sync waits onto standalone NoOps (several instruction
    structs only have one wait slot; walrus hard-errors otherwise)."""
    import concourse.mybir as mybir

    for f in nc.m.functions:
        for blk in f.blocks:
            out = []
            changed = False
            for i in blk.instructions:
                si = i.sync_info
                tname = type(i).__name__
                if (si is not None and si.on_wait
                        and len(si.on_wait) > limit
                        and tname not in ("InstEventSemaphore",)):
                    waits = list(si.on_wait)
                    keep = waits[-limit:]
                    spill = waits[:-limit]
                    for k, w in enumerate(spill):
                        nop = mybir.InstNoOp(
                            name=f"{i.name}-sw{k}", ins=[], outs=[])
                        nop.engine = i.engine
                        nop.sync_info = mybir.SyncInfo(
                            on_wait=[w], on_update=[])
                        out.append(nop)
                    i.sync_info = mybir.SyncInfo(
                        on_wait=keep, on_update=list(si.on_update or []))
                    changed = True
                out.append(i)
            if changed:
                blk.instructions = out


_NC_CACHE = None


def _get_program(split=True):
    global _NC_CACHE
    if _NC_CACHE is None:
        _NC_CACHE = _build_program()
    if split and not getattr(_NC_CACHE, "_waits_split", False):
        _split_sync_waits(_NC_CACHE)
        _NC_CACHE._waits_split = True
    return _NC_CACHE


def _core_inputs(core, outs, anchors, threshold):
    """Build the DRAM input map for one core. Pure data marshaling."""
    m = {}
    for s, x_full in zip(SCFG, outs):
        n = s["name"]
        HW, nblk = s["HW"], s["nblk"]
        x = np.asarray(
            x_full[core * B_LOC:(core + 1) * B_LOC], dtype=np.float32
        ).reshape(B_LOC, 255, HW)
        xp = np.zeros((B_LOC, 255, nblk * 128), np.float32)
        xp[:, :, :HW] = x
        # [b, c, k, p] -> [p, b, k, c]
        m[f"x{n}"] = np.ascontiguousarray(
            xp.reshape(B_LOC, 255, nblk, 128).transpose(3, 0, 2, 1)
        ).reshape(128, -1)
    cst = np.zeros((128, CST_COLS), np.float32)
    cst[:, _CST_W8:_CST_W8 + 8] = (8.0 - np.arange(8))[None, :]
    cst[:, _CST_W10:_CST_W10 + 10] = (10.0 - np.arange(10))[None, :]
    cst[:, _CST_THR] = np.float32(np.asarray(threshold)[0])
    for s, anch in zip(SCFG, anchors):
        n = s["name"]
        HW, nblk, W, stride = s["HW"], s["nblk"], s["W"], s["stride"]
        off = _CST_SC[n]
        cell = (np.arange(nblk)[None, :] * 128
                + np.arange(128)[:, None])  # [p, k]
        valid = cell < HW
        gx = np.where(valid, (cell % W) * stride, 0.0).astype(np.float32)
        gy = np.where(valid, (cell // W) * stride, 0.0).astype(np.float32)
        cst[:, off:off + 4 * nblk] = np.tile(gx, (1, B_LOC))
        cst[:, off + 4 * nblk:off + 8 * nblk] = np.tile(gy, (1, B_LOC))
        cst[:, off + 8 * nblk:off + 8 * nblk + 6] = np.asarray(
            anch, np.float32).reshape(6)[None, :]
    m["cst"] = cst
    return m


def _assemble_core(res):
    """Interleave one core's packed outputs into reference row order."""
    per_scale = []
    for s in SCFG:
        n = s["name"]
        HW, nblk = s["HW"], s["nblk"]
        o = res[f"opack{n}"].reshape(128, B_LOC, nblk, 3, 6)
        rows = (o.transpose(1, 2, 0, 3, 4)
                .reshape(B_LOC, nblk * 128, 3, 6)[:, :HW]
                .reshape(B_LOC * HW * 3, 6))
        per_scale.append(rows)
    return per_scale


def kernel(output_13, output_26, output_52, anchors_13, anchors_26,
           anchors_52, threshold):
    from concourse.bass_utils import run_bass_kernel_spmd

    nc = _get_program()
    outs = (np.asarray(output_13), np.asarray(output_26),
            np.asarray(output_52))
    anchors = (np.asarray(anchors_13), np.asarray(anchors_26),
               np.asarray(anchors_52))
    thr = np.asarray(threshold)

    in_maps = [_core_inputs(cc, outs, anchors, thr) for cc in range(N_CORES)]
    r = run_bass_kernel_spmd(nc, in_maps, list(range(N_CORES)))
    per_core = [_assemble_core(r.results[cc]) for cc in range(N_CORES)]
    blocks = []
    for si in range(3):
        blocks.append(np.concatenate([per_core[cc][si]
                                      for cc in range(N_CORES)], axis=0))
    return np.concatenate(blocks, axis=0).astype(np.float32)


# revision 28
# speedup vs baseline: 1.0167x; 1.0167x over previous
"""YOLOv3-style detection decode kernel for Trainium2 (8 NeuronCores).

Data-parallel over batch (32 -> 4 per core). Host marshals each core's head
tensors into a cells-on-partitions layout x[p, (b k a c)] (cell = k*128+p,
c = 85 attrs per anchor); since 3 anchors * 85 = 255 = the channel count,
(b, k, a) collapse into one free dim Z and the device needs no transposes:

  - argmax over the 80 classes per (cell, anchor) via two segmented DVE
    reductions: phase-maxes p8[j] = max_g x[8g+j] and group-maxes
    q10[g] = max_j x[8g+j] (one tensor_reduce each per scale). The class
    index is 8*g* + j*, with g*/j* recovered by an is_ge-against-max
    compare and a descending-weight max (ties break toward the FIRST
    index, matching jnp.argmax).
  - box decode reads strided views of the same tiles (exp/scale on ACT).
  - outputs are packed [p, b, k, a, 6] per scale; the host re-interleaves.
"""

import sys

import numpy as np

if "/opt/trn_rl_repo" not in sys.path:
    sys.path.insert(0, "/opt/trn_rl_repo")

NUM_ATTRS = 85
B_LOC = 4  # batches per core (32 / 8)
N_CORES = 8

# (name, H, stride)
_SCALES = (
    ("13", 13, 32.0),
    ("26", 26, 16.0),
    ("52", 52, 8.0),
)


def _scale_cfg():
    cfgs = []
    for name, H, stride in _SCALES:
        HW = H * H
        nblk = -(-HW // 128)
        cfgs.append(dict(name=name, H=H, W=H, HW=HW, stride=stride,
                         nblk=nblk, HWp=nblk * 128))
    return cfgs


SCFG = _scale_cfg()

# consts layout: w8(8) | w10(10) | thr(1) | per scale: gx4(4nb) gy4(4nb) anch(6)
_CST_W8 = 0
_CST_W10 = 8
_CST_THR = 18
_CST_SC = {}
_off = 19
for _s in SCFG:
    _CST_SC[_s["name"]] = _off
    _off += 8 * _s["nblk"] + 6
CST_COLS = _off


def _build_program():
    import concourse.bass as bass
    import concourse.mybir as mybir
    from concourse.tile import TileContext

    f32 = mybir.dt.float32
    bf16 = mybir.dt.bfloat16
    Alu = mybir.AluOpType
    Act = mybir.ActivationFunctionType
    X = mybir.AxisListType.X

    nc = bass.Bass(trn_type="TRN2")

    xin = {}
    opk = {}
    for s in SCFG:
        n = s["name"]
        xin[n] = nc.declare_dram_parameter(
            f"x{n}", [128, B_LOC * s["nblk"] * 255], f32, False)
        opk[n] = nc.declare_dram_parameter(
            f"opack{n}", [128, B_LOC * s["nblk"] * 18], f32, True)
    cst_p = nc.declare_dram_parameter("cst", [128, CST_COLS], f32, False)

    with TileContext(nc) as tc:
        from contextlib import ExitStack
        with ExitStack() as ctx:
            cpool = ctx.enter_context(tc.tile_pool(name="consts", bufs=1))
            x52pool = ctx.enter_context(tc.tile_pool(name="x52", bufs=2))
            # fold scratch: consumed only by the (serial) DVE queue, so a
            # single buffer per tag costs no parallelism
            fpool = ctx.enter_context(tc.tile_pool(name="folds", bufs=1))

            # x13 first, in two halves: its consumers idle at kernel start,
            # so the first half landing early lets the DVE start sooner.
            # Consts are only needed later (mask/cx); they stream after.
            _c13 = B_LOC * SCFG[0]["nblk"] * 255
            xt13 = cpool.tile([128, _c13], f32, tag="xt13", name="xt13")
            nc.sync.dma_start(out=xt13[:, 0:_c13 // 2],
                              in_=xin["13"][:, 0:_c13 // 2])
            nc.sync.dma_start(out=xt13[:, _c13 // 2:],
                              in_=xin["13"][:, _c13 // 2:])
            cstt = cpool.tile([128, CST_COLS], f32, tag="cst", name="cstt")
            nc.sync.dma_start(out=cstt[:, :], in_=cst_p[:, :])
            thr = cstt[:, _CST_THR:_CST_THR + 1]

            # bf16 copies of the index weights (2-byte dtype enables the
            # DVE 2x mode for the small extraction ops)
            w8c = cpool.tile([128, 8], bf16, tag="w8c", name="w8c")
            nc.vector.tensor_copy(out=w8c[:, :],
                                  in_=cstt[:, _CST_W8:_CST_W8 + 8])
            w10c = cpool.tile([128, 10], bf16, tag="w10c", name="w10c")
            nc.vector.tensor_copy(out=w10c[:, :],
                                  in_=cstt[:, _CST_W10:_CST_W10 + 10])

            def scale_tiles(s):
                n, nb = s["name"], s["nblk"]
                Z = B_LOC * nb * 3
                t = {}
                for key, w in (("p8", 8), ("q10", 10), ("m", 1),
                               ("ts", 1), ("mask", 1), ("ex", 2), ("wh", 2),
                               ("cx", 1), ("cy", 1), ("opk", 6)):
                    t[key] = cpool.tile([128, Z * w], f32, tag=f"{key}{n}",
                                        name=f"{key}{n}")
                for key, w in (("eq8", 8), ("eq10", 10), ("ew8", 8),
                               ("ew10", 10), ("r8", 1), ("r10", 1),
                               ("rs8", 4), ("rs10", 5)):
                    t[key] = cpool.tile([128, Z * w], bf16, tag=f"{key}{n}",
                                        name=f"{key}{n}")
                t["m2"] = cpool.tile([128, Z * 4], f32, tag=f"m2{n}",
                                     name=f"m2{n}")
                t["bx"] = cpool.tile([128, Z * 4], f32, tag=f"bx{n}",
                                     name=f"bx{n}")
                return t

            def class_reduces(s, t, xtv, lo):
                """Phase/group reduces + conf + cx/cy for one input view.

                xtv: [p, z, c=85] view (z spans any (b k a) range); lo is the
                z offset of this view within the scale's scratch tiles.
                """
                n, nb, stride = s["name"], s["nblk"], s["stride"]
                off = _CST_SC[n]
                zc = xtv.shape[1]  # cells*anchors covered by this view
                hi = lo + zc

                cls = xtv[:, :, 5:85]
                p8v = t["p8"][:, lo * 8:hi * 8].rearrange(
                    "p (z j) -> p z j", j=8)
                q10v = t["q10"][:, lo * 10:hi * 10].rearrange(
                    "p (z g) -> p z g", g=10)

                # Tournament folds (tensor_tensor reads two streams/cycle,
                # so folds cost half a single-stream tensor_reduce; all
                # slices are stride-1 in the inner dim).
                # phase-max p8[j] = max_g cls[8g+j]: fold the group axis.
                f1 = fpool.tile([128, zc * 40], f32, tag="f1", name="f1")
                f1v = f1[:, :].rearrange("p (z c) -> p z c", c=40)
                nc.vector.tensor_tensor(out=f1v, in0=cls[:, :, 0:40],
                                        in1=cls[:, :, 40:80], op=Alu.max)
                f2 = fpool.tile([128, zc * 16], f32, tag="f2", name="f2")
                f2v = f2[:, :].rearrange("p (z c) -> p z c", c=16)
                nc.vector.tensor_tensor(out=f2v, in0=f1v[:, :, 0:16],
                                        in1=f1v[:, :, 16:32], op=Alu.max)
                f3 = fpool.tile([128, zc * 8], f32, tag="f3", name="f3")
                f3v = f3[:, :].rearrange("p (z c) -> p z c", c=8)
                nc.vector.tensor_tensor(out=f3v, in0=f2v[:, :, 0:8],
                                        in1=f2v[:, :, 8:16], op=Alu.max)
                nc.vector.tensor_tensor(out=p8v, in0=f3v,
                                        in1=f1v[:, :, 32:40], op=Alu.max)
                # group-max q10[g] = max_j cls[8g+j]: fold within groups.
                cg = cls.rearrange("p z (g j) -> p z g j", g=10, j=8)
                y1 = fpool.tile([128, zc * 40], f32, tag="y1", name="y1")
                y1v = y1[:, :].rearrange("p (z g j) -> p z g j", g=10, j=4)
                nc.vector.tensor_tensor(out=y1v, in0=cg[:, :, :, 0:4],
                                        in1=cg[:, :, :, 4:8], op=Alu.max)
                y1z = y1[:, :].rearrange("p (w j) -> p w j", j=4)
                y2 = fpool.tile([128, zc * 20], f32, tag="y2", name="y2")
                y2z = y2[:, :].rearrange("p (w j) -> p w j", j=2)
                nc.vector.tensor_tensor(out=y2z, in0=y1z[:, :, 0:2],
                                        in1=y1z[:, :, 2:4], op=Alu.max)
                nc.vector.tensor_tensor(
                    out=t["q10"][:, lo * 10:hi * 10],
                    in0=y2z[:, :, 0:1].squeeze(2),
                    in1=y2z[:, :, 1:2].squeeze(2), op=Alu.max)

                conf = xtv[:, :, 0:1].squeeze(2)
                maskv = t["mask"][:, lo:hi]
                nc.vector.tensor_single_scalar(
                    out=maskv, in_=conf, scalar=thr, op=Alu.is_gt)
                opkz = t["opk"][:, lo * 6:hi * 6].rearrange(
                    "p (z q) -> p z q", q=6)
                nc.vector.tensor_tensor(
                    out=opkz[:, :, 0:1].squeeze(2), in0=conf, in1=maskv,
                    op=Alu.mult)

                # exp(tw,th) on ACT; cx/cy on DVE
                exv = t["ex"][:, lo * 2:hi * 2].rearrange(
                    "p (z e) -> p z e", e=2)
                nc.scalar.activation(out=exv, in_=xtv[:, :, 3:5],
                                     func=Act.Exp)
                # gx4/gy4 are host-replicated over batches, so a [p, zc/3, 3]
                # broadcast view always lines up with this z range.
                gx = cstt[:, off + lo // 3:off + hi // 3]
                gy = cstt[:, off + 4 * nb + lo // 3:off + 4 * nb + hi // 3]
                gxb = gx.unsqueeze(2).broadcast_to([128, zc // 3, 3])
                gyb = gy.unsqueeze(2).broadcast_to([128, zc // 3, 3])
                tx = xtv[:, :, 1:2].squeeze(2).rearrange(
                    "p (w a) -> p w a", a=3)
                ty = xtv[:, :, 2:3].squeeze(2).rearrange(
                    "p (w a) -> p w a", a=3)
                cxv = t["cx"][:, lo:hi].rearrange("p (w a) -> p w a", a=3)
                cyv = t["cy"][:, lo:hi].rearrange("p (w a) -> p w a", a=3)
                nc.vector.scalar_tensor_tensor(
                    out=cxv, in0=tx, scalar=stride, in1=gxb,
                    op0=Alu.mult, op1=Alu.add)
                nc.vector.scalar_tensor_tensor(
                    out=cyv, in0=ty, scalar=stride, in1=gyb,
                    op0=Alu.mult, op1=Alu.add)

            def per_scale(s, t):
                """All-batch epilogue on compact scratch tiles."""
                n, nb = s["name"], s["nblk"]
                off = _CST_SC[n]
                anch = cstt[:, off + 8 * nb:off + 8 * nb + 6]
                Z = B_LOC * nb * 3

                def zv(tile, w):
                    return tile[:, :].rearrange("p (z q) -> p z q", q=w)

                p8 = zv(t["p8"], 8)
                q10 = zv(t["q10"], 10)
                eq8 = zv(t["eq8"], 8)
                eq10 = zv(t["eq10"], 10)
                ew8 = zv(t["ew8"], 8)
                ew10 = zv(t["ew10"], 10)
                m = t["m"][:, :]
                r8 = t["r8"][:, :]
                r10 = t["r10"][:, :]
                tsv = t["ts"][:, :]
                mask = t["mask"][:, :]
                opkq = zv(t["opk"], 6)

                # m = max over phases, via TT folds (8 -> 4 -> 2 -> 1)
                m2 = zv(t["m2"], 4)
                nc.vector.tensor_tensor(out=m2, in0=p8[:, :, 0:4],
                                        in1=p8[:, :, 4:8], op=Alu.max)
                m2z = t["m2"][:, :].rearrange("p (w c) -> p w c", c=2)
                nc.vector.tensor_tensor(out=m2z[:, :, 0:1].squeeze(2),
                                        in0=m2z[:, :, 0:1].squeeze(2),
                                        in1=m2z[:, :, 1:2].squeeze(2),
                                        op=Alu.max)
                m4 = zv(t["m2"], 4)
                nc.vector.tensor_tensor(out=m, in0=m4[:, :, 0:1].squeeze(2),
                                        in1=m4[:, :, 2:3].squeeze(2),
                                        op=Alu.max)
                mb8 = m.unsqueeze(2).broadcast_to([128, Z, 8])
                mb10 = m.unsqueeze(2).broadcast_to([128, Z, 10])
                w8b = w8c[:, :].unsqueeze(1).broadcast_to([128, Z, 8])
                w10b = w10c[:, :].unsqueeze(1).broadcast_to([128, Z, 10])

                # j* / g* via descending-weight max (first-index tie-break);
                # eq/weights/r are exact small values -> bf16 (2x DVE mode)
                nc.vector.tensor_tensor(out=eq8, in0=p8, in1=mb8,
                                        op=Alu.is_ge)
                nc.vector.tensor_tensor(out=ew8, in0=eq8, in1=w8b,
                                        op=Alu.mult)
                rs8 = zv(t["rs8"], 4)
                nc.vector.tensor_tensor(out=rs8, in0=ew8[:, :, 0:4],
                                        in1=ew8[:, :, 4:8], op=Alu.max)
                rs8z = t["rs8"][:, :].rearrange("p (w c) -> p w c", c=2)
                nc.vector.tensor_tensor(out=rs8z[:, :, 0:1].squeeze(2),
                                        in0=rs8z[:, :, 0:1].squeeze(2),
                                        in1=rs8z[:, :, 1:2].squeeze(2),
                                        op=Alu.max)
                nc.vector.tensor_tensor(out=r8, in0=rs8[:, :, 0:1].squeeze(2),
                                        in1=rs8[:, :, 2:3].squeeze(2),
                                        op=Alu.max)
                nc.vector.tensor_tensor(out=eq10, in0=q10, in1=mb10,
                                        op=Alu.is_ge)
                nc.vector.tensor_tensor(out=ew10, in0=eq10, in1=w10b,
                                        op=Alu.mult)
                rs10 = zv(t["rs10"], 5)
                nc.vector.tensor_tensor(out=rs10, in0=ew10[:, :, 0:5],
                                        in1=ew10[:, :, 5:10], op=Alu.max)
                nc.vector.tensor_tensor(out=rs10[:, :, 0:2],
                                        in0=rs10[:, :, 0:2],
                                        in1=rs10[:, :, 2:4], op=Alu.max)
                nc.vector.tensor_tensor(out=rs10[:, :, 0:1].squeeze(2),
                                        in0=rs10[:, :, 0:1].squeeze(2),
                                        in1=rs10[:, :, 1:2].squeeze(2),
                                        op=Alu.max)
                nc.vector.tensor_tensor(out=r10,
                                        in0=rs10[:, :, 0:1].squeeze(2),
                                        in1=rs10[:, :, 4:5].squeeze(2),
                                        op=Alu.max)
                # idx = 88 - 8*r10 - r8 ; cls_m = (ts + 88) * mask
                nc.vector.scalar_tensor_tensor(
                    out=tsv, in0=r10, scalar=-8.0, in1=r8,
                    op0=Alu.mult, op1=Alu.subtract)
                nc.vector.scalar_tensor_tensor(
                    out=opkq[:, :, 5:6].squeeze(2), in0=tsv, scalar=88.0,
                    in1=mask, op0=Alu.add, op1=Alu.mult)

                # boxes: wh = anch * exp (ACT) -> x1/y1/x2/y2 -> masked
                exw = t["ex"][:, :].rearrange("p (w a e) -> p w a e",
                                              a=3, e=2)
                whw = t["wh"][:, :].rearrange("p (w a e) -> p w a e",
                                              a=3, e=2)
                for a in range(3):
                    for q in range(2):
                        nc.scalar.activation(
                            out=whw[:, :, a, q:q + 1].squeeze(2),
                            in_=exw[:, :, a, q:q + 1].squeeze(2),
                            func=Act.Copy,
                            scale=anch[:, 2 * a + q:2 * a + q + 1])
                whz = zv(t["wh"], 2)
                wx = whz[:, :, 0:1].squeeze(2)
                wy = whz[:, :, 1:2].squeeze(2)
                cx = t["cx"][:, :]
                cy = t["cy"][:, :]
                bx = zv(t["bx"], 4)
                for q, (wv, cv, sc) in enumerate(
                        ((wx, cx, -0.5), (wy, cy, -0.5),
                         (wx, cx, 0.5), (wy, cy, 0.5))):
                    nc.vector.scalar_tensor_tensor(
                        out=bx[:, :, q:q + 1].squeeze(2), in0=wv, scalar=sc,
                        in1=cv, op0=Alu.mult, op1=Alu.add)
                mb4 = mask.unsqueeze(2).broadcast_to([128, Z, 4])
                nc.vector.tensor_tensor(out=opkq[:, :, 1:5], in0=bx,
                                        in1=mb4, op=Alu.mult)
                nc.sync.dma_start(out=opk[n][:, :], in_=t["opk"][:, :])

            # ---- scale 13: two half-batch views of the split DMA ----
            s = SCFG[0]
            t = scale_tiles(s)
            for h in range(2):
                vz = xt13[:, h * _c13 // 2:(h + 1) * _c13 // 2].rearrange(
                    "p (z c) -> p z c", c=85)
                class_reduces(s, t, vz, h * 2 * s["nblk"] * 3)
            per_scale(s, t)

            # ---- scale 26: one static tile, all batches at once ----
            s = SCFG[1]
            t = scale_tiles(s)
            xt26 = cpool.tile([128, B_LOC * s["nblk"] * 255], f32,
                              tag="xt26", name="xt26")
            nc.sync.dma_start(out=xt26[:, :], in_=xin["26"][:, :])
            vz = xt26[:, :].rearrange("p (z c) -> p z c", c=85)
            class_reduces(s, t, vz, 0)
            per_scale(s, t)

            # ---- scale 52: per-batch pipelined input tiles ----
            s = SCFG[2]
            t = scale_tiles(s)
            nb = s["nblk"]
            for b in range(B_LOC):
                xtb = x52pool.tile([128, nb * 255], f32, tag="x52",
                                   name="x52b")
                nc.sync.dma_start(
                    out=xtb[:, :],
                    in_=xin[s["name"]][:, b * nb * 255:(b + 1) * nb * 255])
                vz = xtb[:, :].rearrange("p (z c) -> p z c", c=85)
                class_reduces(s, t, vz, b * nb * 3)
            per_scale(s, t)

    return nc


def _split_sync_waits(nc, limit=1):
    """Move overflow sync waits onto standalone NoOps (several instruction
    structs only have one wait slot; walrus hard-errors otherwise)."""
    import concourse.mybir as mybir

    for f in nc.m.functions:
        for blk in f.blocks:
            out = []
            changed = False
            for i in blk.instructions:
                si = i.sync_info
                tname = type(i).__name__
                if (si is not None and si.on_wait
                        and len(si.on_wait) > limit
                        and tname not in ("InstEventSemaphore",)):
                    waits = list(si.on_wait)
                    keep = waits[-limit:]
                    spill = waits[:-limit]
                    for k, w in enumerate(spill):
                        nop = mybir.InstNoOp(
                            name=f"{i.name}-sw{k}", ins=[], outs=[])
                        nop.engine = i.engine
                        nop.sync_info = mybir.SyncInfo(
                            on_wait=[w], on_update=[])
                        out.append(nop)
                    i.sync_info = mybir.SyncInfo(
                        on_wait=keep, on_update=list(si.on_update or []))
                    changed = True
                out.append(i)
            if changed:
                blk.instructions = out


_NC_CACHE = None


def _get_program(split=True):
    global _NC_CACHE
    if _NC_CACHE is None:
        _NC_CACHE = _build_program()
    if split and not getattr(_NC_CACHE, "_waits_split", False):
        _split_sync_waits(_NC_CACHE)
        _NC_CACHE._waits_split = True
    return _NC_CACHE


def _core_inputs(core, outs, anchors, threshold):
    """Build the DRAM input map for one core. Pure data marshaling."""
    m = {}
    for s, x_full in zip(SCFG, outs):
        n = s["name"]
        HW, nblk = s["HW"], s["nblk"]
        x = np.asarray(
            x_full[core * B_LOC:(core + 1) * B_LOC], dtype=np.float32
        ).reshape(B_LOC, 255, HW)
        xp = np.zeros((B_LOC, 255, nblk * 128), np.float32)
        xp[:, :, :HW] = x
        # [b, c, k, p] -> [p, b, k, c]
        m[f"x{n}"] = np.ascontiguousarray(
            xp.reshape(B_LOC, 255, nblk, 128).transpose(3, 0, 2, 1)
        ).reshape(128, -1)
    cst = np.zeros((128, CST_COLS), np.float32)
    cst[:, _CST_W8:_CST_W8 + 8] = (8.0 - np.arange(8))[None, :]
    cst[:, _CST_W10:_CST_W10 + 10] = (10.0 - np.arange(10))[None, :]
    cst[:, _CST_THR] = np.float32(np.asarray(threshold)[0])
    for s, anch in zip(SCFG, anchors):
        n = s["name"]
        HW, nblk, W, stride = s["HW"], s["nblk"], s["W"], s["stride"]
        off = _CST_SC[n]
        cell = (np.arange(nblk)[None, :] * 128
                + np.arange(128)[:, None])  # [p, k]
        valid = cell < HW
        gx = np.where(valid, (cell % W) * stride, 0.0).astype(np.float32)
        gy = np.where(valid, (cell // W) * stride, 0.0).astype(np.float32)
        cst[:, off:off + 4 * nblk] = np.tile(gx, (1, B_LOC))
        cst[:, off + 4 * nblk:off + 8 * nblk] = np.tile(gy, (1, B_LOC))
        cst[:, off + 8 * nblk:off + 8 * nblk + 6] = np.asarray(
            anch, np.float32).reshape(6)[None, :]
    m["cst"] = cst
    return m


def _assemble_core(res):
    """Interleave one core's packed outputs into reference row order."""
    per_scale = []
    for s in SCFG:
        n = s["name"]
        HW, nblk = s["HW"], s["nblk"]
        o = res[f"opack{n}"].reshape(128, B_LOC, nblk, 3, 6)
        rows = (o.transpose(1, 2, 0, 3, 4)
                .reshape(B_LOC, nblk * 128, 3, 6)[:, :HW]
                .reshape(B_LOC * HW * 3, 6))
        per_scale.append(rows)
    return per_scale


def kernel(output_13, output_26, output_52, anchors_13, anchors_26,
           anchors_52, threshold):
    from concourse.bass_utils import run_bass_kernel_spmd

    nc = _get_program()
    outs = (np.asarray(output_13), np.asarray(output_26),
            np.asarray(output_52))
    anchors = (np.asarray(anchors_13), np.asarray(anchors_26),
               np.asarray(anchors_52))
    thr = np.asarray(threshold)

    in_maps = [_core_inputs(cc, outs, anchors, thr) for cc in range(N_CORES)]
    r = run_bass_kernel_spmd(nc, in_maps, list(range(N_CORES)))
    per_core = [_assemble_core(r.results[cc]) for cc in range(N_CORES)]
    blocks = []
    for si in range(3):
        blocks.append(np.concatenate([per_core[cc][si]
                                      for cc in range(N_CORES)], axis=0))
    return np.concatenate(blocks, axis=0).astype(np.float32)
